# revision 33
# baseline (speedup 1.0000x reference)
"""Trainium2 Bass kernel for nn_DA_conv: per-sample dynamic depthwise 3x3 conv
(+LeakyReLU) followed by a 1x1 pointwise conv, with the 3x3 kernels produced by
a small per-sample MLP.

Strategy (8 NeuronCores, pure batch data-parallel, 2 samples per core):
  - SBUF partition p = (sample s = p//64, channel c = p%64); the 2-sample
    feature map lives resident in SBUF, zero-padded ON THE HOST so the DMA in
    is fully contiguous (1 descriptor per partition per chunk).
  - Depthwise conv split across engines by image-row region:
      * PE rows:  9 PSUM-accumulating full-128-partition diagonal matmuls per
        512-px tile; Prelu evacuation (1024 px) on Act.
      * DP rows:  per-tap products on DVE (tensor_scalar_mul, 4x bf16 mode),
        add tree on Pool (tensor_tensor), LeakyReLU on Act.
      * D rows:   products + add tree fully on DVE, LeakyReLU on Act.
  - 1x1 conv = block-diagonal [128x128] bf16 matmuls; PSUM evacuated by Act
    with the bias add fused (Identity + per-partition bias), written bf16.
  - All DMA transfers are engine-time in this machine model, so x chunks are
    split between the SP and Act queues in consumption order; output DMAs
    ride SP.
"""

import sys

sys.path.insert(0, "/opt/trn_rl_repo")

from contextlib import ExitStack

import numpy as np

import concourse.bacc as bacc
import concourse.bass as bass
import concourse.mybir as mybir
import concourse.tile as tile

S = 2            # samples per core
C = 64           # channels
H = W = 128      # spatial
KK = 3           # conv kernel size
NCORES = 8
RS = 132         # padded row stride in elements
RP = H + 2       # padded row count (top/bottom halo)
XFREE = RP * RS  # padded image elements per partition

f32 = mybir.dt.float32
bf16 = mybir.dt.bfloat16
i32 = mybir.dt.int32

LRELU = mybir.ActivationFunctionType.Prelu
TAPS = [(di, dj) for di in range(KK) for dj in range(KK)]  # t = di*3 + dj

# ---- region assignment (rows of the 128-row image) ----
PE_GROUPS = [4 * g for g in range(17)]              # rows 0..67
DP_CHUNKS = [(68, 4), (72, 12), (84, 12), (96, 8)]  # DVE muls + Pool adds
D_CHUNKS = [(104, 12), (116, 12)]                   # all-DVE
ADD_TREE = [  # (dst, src) pairs over 9 product slots; acc ends in slot 0
    (0, 1), (2, 3), (4, 5), (6, 7), (0, 2), (4, 6), (0, 4), (0, 8),
]
# x chunks in PADDED row space (padded row pr holds image row pr-1), all on
# the SP queue, ordered for earliest consumer.
X_CHUNKS_SP = [(0, 18), (46, 28), (74, 16), (18, 28), (90, 16), (106, 24)]


def build_program() -> bass.Bass:
    nc = bacc.Bacc("TRN2", target_bir_lowering=False, debug=False)

    x_d = nc.dram_tensor("xpad", [S * C, XFREE], bf16, kind="ExternalInput").ap()
    # dT, wk1t, wk2td concatenated along the free dim (all partition-64):
    # cols [0,2) = dT, [2,66) = wk1t, [66,1218) = wk2td where
    # wk2td[j, t*128 + s*64 + c] = Wk2[c*9 + t, j]
    NSM = S + C + KK * KK * 2 * C
    sm_d = nc.dram_tensor("smalls", [C, NSM], bf16, kind="ExternalInput").ap()
    # block-diagonal 1x1 weights: wcb[(s,ci),(s,co)] = Wc[co,ci]
    wcb_d = nc.dram_tensor("wcb", [2 * C, 2 * C], bf16, kind="ExternalInput").ap()
    bc_d = nc.dram_tensor("bc2", [2 * C, 1], f32, kind="ExternalInput").ap()
    out_d = nc.dram_tensor("out", [S * C, H * W], bf16, kind="ExternalOutput").ap()

    with tile.TileContext(nc) as tc, ExitStack() as ctx:
        _body(ctx, tc, x_d, sm_d, wcb_d, bc_d, out_d)
    nc.compile()
    return nc


def _body(ctx, tc, x_d, sm_d, wcb_d, bc_d, out_d):
    nc = tc.nc
    const = ctx.enter_context(tc.tile_pool(name="const", bufs=1))
    xpool = ctx.enter_context(tc.tile_pool(name="xs", bufs=1))
    dgp = ctx.enter_context(tc.tile_pool(name="dg", bufs=1))
    dpprod = ctx.enter_context(tc.tile_pool(name="dpprod", bufs=3))
    dprod = ctx.enter_context(tc.tile_pool(name="dprod", bufs=1))
    accp = ctx.enter_context(tc.tile_pool(name="acc", bufs=2))
    ostg = ctx.enter_context(tc.tile_pool(name="ostg", bufs=4))
    pdw = ctx.enter_context(tc.tile_pool(name="pdw", bufs=2, space="PSUM"))
    po2 = ctx.enter_context(tc.tile_pool(name="po2", bufs=2, space="PSUM"))

    # ---------------- input loads ----------------
    # MLP weights (bf16, one merged DMA) on the Act queue first (they gate
    # kcols/diag); all of x plus wcb/bc2 on SP in consumption order.
    NSM = S + C + KK * KK * 2 * C
    smalls = const.tile([C, NSM], bf16)
    nc.scalar.dma_start(smalls[:, :], sm_d)

    def dts():
        return smalls[:, 0:S]

    def wk1t():
        return smalls[:, S : S + C]

    def wk2td(t):
        o = S + C + t * 128
        return smalls[:, o : o + 128]

    xs = xpool.tile([128, XFREE], bf16)

    def load_x(engine, pr0, npr):
        engine.dma_start(
            xs[:, pr0 * RS : (pr0 + npr) * RS], x_d[:, pr0 * RS : (pr0 + npr) * RS]
        )

    for chunk in X_CHUNKS_SP[:2]:
        load_x(nc.sync, *chunk)

    # ---------------- kernel-generating MLP ----------------
    hid_ps = po2.tile([C, S], f32, tag="oo")
    nc.tensor.matmul(
        hid_ps[:, :], lhsT=wk1t(), rhs=dts(), start=True, stop=True,
    )
    hid_sb = const.tile([C, S], bf16)
    nc.scalar.activation(hid_sb[:, :], hid_ps[:, :], LRELU, alpha=0.1)

    # kern tap columns: kcols[s*64+c, t] = kern[s, c*9+t].
    # All 9 tap matmuls write one psum tile; two strided copies pick the
    # sample-matched column per partition half.
    kps = po2.tile([2 * C, 2 * KK * KK], f32, tag="oo")
    for t in range(KK * KK):
        nc.tensor.matmul(
            kps[:, 2 * t : 2 * t + 2],
            lhsT=wk2td(t),
            rhs=hid_sb[:, :],
            start=True, stop=True,
        )
    kcols = const.tile([2 * C, KK * KK], f32)
    kps3 = kps[:, :].rearrange("p (t s) -> p t s", s=2)
    nc.vector.tensor_copy(kcols[0:C, :], kps3[0:C, :, 0])
    nc.vector.tensor_copy(kcols[C : 2 * C, :], kps3[C : 2 * C, :, 1])

    for chunk in X_CHUNKS_SP[2:]:
        load_x(nc.sync, *chunk)
    wcb = const.tile([2 * C, 2 * C], bf16)
    nc.sync.dma_start(wcb[:, :], wcb_d)
    bc2 = const.tile([2 * C, 1], f32)
    nc.sync.dma_start(bc2[:, :], bc_d)

    # identity -> per-tap diagonal weight matrices diag[:, t*128:(t+1)*128]
    id_i = const.tile([128, 128], i32)
    nc.gpsimd.iota(id_i[:, :], pattern=[[1, 128]], base=0, channel_multiplier=-1)
    idf = const.tile([128, 128], f32)
    nc.vector.tensor_scalar(idf[:, :], id_i[:, :], 0, None, mybir.AluOpType.is_equal)
    diag = const.tile([128, KK * KK * 128], bf16)
    for t in range(KK * KK):
        nc.vector.tensor_scalar_mul(
            diag[:, t * 128 : (t + 1) * 128], idf[:, :], kcols[:, t : t + 1]
        )

    # ---------------- main loop ----------------
    xrows = xs[:, :].rearrange("p (r w) -> p r w", w=RS)

    def win(r0, nr, di, dj):
        # image rows r0..r0+nr-1 under tap (di,dj); padded row r0+di covers
        # image row r0+di-1 (the +1 pad offset cancels the tap's -1).
        return xrows[:, r0 + di : r0 + di + nr, dj : dj + W]

    dg = {}  # image row -> (tile, px offset) for 4-row (512 px) slices

    def set_dg(r0, nr, tilev, base=0):
        for i in range(nr // 4):
            dg[r0 + 4 * i] = (tilev, base + 512 * i)

    pcur = {"t": None}

    def pe_group(gi, r0):
        # two groups share one [128,1024] psum tile (2 banks)
        if gi % 2 == 0:
            pcur["t"] = pdw.tile([128, 1024], f32, tag="pdw", name=f"pdw{r0}")
        P = pcur["t"]
        half = 512 * (gi % 2)
        for t, (di, dj) in enumerate(TAPS):
            nc.tensor.matmul(
                P[:, half : half + 512],
                lhsT=diag[:, t * 128 : (t + 1) * 128],
                rhs=win(r0, 4, di, dj),
                start=(t == 0), stop=(t == KK * KK - 1),
            )
        if gi % 2 == 1 or gi == len(PE_GROUPS) - 1:
            npx = half + 512
            rbase = r0 - 4 * (gi % 2)
            D = dgp.tile([128, npx], bf16, name=f"dpe{rbase}")
            nc.scalar.activation(D[:, 0:npx], P[:, 0:npx], LRELU, alpha=0.1)
            set_dg(rbase, npx // 128, D)

    def dve_muls(r0, nr, pool):
        px = nr * W
        prod = pool.tile([128, 9 * px], bf16, tag="prod", name=f"prod{r0}")
        p3 = prod[:, :].rearrange("p (t x) -> p t x", x=px)
        for t, (di, dj) in enumerate(TAPS):
            o = p3[:, t, :].rearrange("p (r w) -> p r w", w=W)
            nc.vector.tensor_scalar_mul(o, win(r0, nr, di, dj), kcols[:, t : t + 1])
        return p3

    def adds_and_lrelu(eng, p3, r0, nr, tag, lrelu_dve=False):
        px = nr * W
        for dst, src in ADD_TREE[:-1]:
            eng.tensor_tensor(
                p3[:, dst, :], p3[:, dst, :], p3[:, src, :], op=mybir.AluOpType.add
            )
        acc = accp.tile([128, px], bf16, tag=tag, name=f"acc{r0}")
        eng.tensor_tensor(
            acc[:, :], p3[:, 0, :], p3[:, 8, :], op=mybir.AluOpType.add
        )
        D = dgp.tile([128, px], bf16, name=f"dd{r0}")
        if lrelu_dve:
            # lrelu(v) = max(v, 0.1v) on DVE keeps the chain on one engine
            nc.vector.scalar_tensor_tensor(
                D[:, :], acc[:, :], 0.1, acc[:, :],
                op0=mybir.AluOpType.mult, op1=mybir.AluOpType.max,
            )
        else:
            nc.scalar.activation(D[:, :], acc[:, :], LRELU, alpha=0.1)
        set_dg(r0, nr, D)

    # --- 1x1 span (8 rows = 1024 px) + bias evac; out DMA per span pair ---
    ost_tiles = {}
    ost_done = {}

    def span_1x1(s, evac_dve=False, out_pool=False):
        r0 = 8 * s
        O = po2.tile([128, 1024], f32, tag="oo", name=f"o2{s}")
        for h in range(2):
            t_, off = dg[r0 + 4 * h]
            nc.tensor.matmul(
                O[:, 512 * h : 512 * (h + 1)],
                lhsT=wcb[:, :], rhs=t_[:, off : off + 512],
                start=True, stop=True,
            )
        q = s // 2
        if q not in ost_tiles:
            ost_tiles[q] = ostg.tile([128, 2048], bf16, tag="ostg", name=f"ostg{q}")
            ost_done[q] = 0
        z = ost_tiles[q]
        zsl = z[:, 1024 * (s % 2) : 1024 * (s % 2 + 1)]
        if evac_dve:
            nc.vector.tensor_scalar_add(zsl, O[:, :], bc2[:, 0:1])
        else:
            nc.scalar.add(zsl, O[:, :], bc2[:, 0:1])
        ost_done[q] += 1
        if q == 7:
            # final pair: DMA each span on its own so the tail chain is short
            nc.sync.dma_start(out_d[:, s * 1024 : (s + 1) * 1024], zsl)
        elif ost_done[q] == 2:
            eng = nc.gpsimd if out_pool else nc.sync
            eng.dma_start(out_d[:, q * 2048 : (q + 1) * 2048], z[:, :])

    # ---------------- schedule (virtual-time ordered emission) ----------
    # Engines execute their streams near-order with a small lookahead, so
    # emit every op at its estimated ready time to avoid head-of-line
    # convoys.  Costs in us, from the TRN2 cost model.
    MUL_C = lambda px: (px * 0.268 + 105) / 1000.0
    ADD_C = lambda px: (px * 0.53 + 105) / 1000.0
    PADD_C = lambda px: (px * 0.833 + 131) / 1000.0

    events = []  # (vtime, seq, emit_fn)
    seq = [0]

    def ev(vt, fn):
        events.append((vt, seq[0], fn))
        seq[0] += 1

    row_ready = {}  # image row (mult of 4) -> vtime its D tile is ready

    # PE dw groups: start ~4.0, ~1.94us each; prelu lands with the pair.
    # Bias the prelu-bearing groups a bit earlier in emission so Act's
    # stream favors the PE dependency chain over queued evac2s.
    vt = 4.0
    for gi, r0 in enumerate(PE_GROUPS):
        vt += 1.94
        bias = -1.5 if (gi % 2 == 1 or gi == len(PE_GROUPS) - 1) else 0.0
        ev(vt + bias, (lambda gi=gi, r0=r0: pe_group(gi, r0)))
        if gi % 2 == 1 or gi == len(PE_GROUPS) - 1:
            rbase = r0 - 4 * (gi % 2)
            for rr in range(rbase, r0 + 4, 4):
                row_ready[rr] = vt + 0.9

    # DVE: DP muls first, then D chunks (muls+adds).  DVE clock starts ~5.
    dvt = 5.0
    for r0, nr in DP_CHUNKS:
        dvt += 9 * MUL_C(nr * W)
        ev(dvt - 9 * MUL_C(nr * W),
           (lambda r0=r0, nr=nr: dp_p3.__setitem__(r0, dve_muls(r0, nr, dpprod))))
    dp_mul_done = {}
    dvt2 = 5.0
    for r0, nr in DP_CHUNKS:
        dvt2 += 9 * MUL_C(nr * W)
        dp_mul_done[r0] = dvt2
    for ci, (r0, nr) in enumerate(D_CHUNKS):
        cost = 9 * MUL_C(nr * W) + 8 * ADD_C(nr * W)
        last = ci == len(D_CHUNKS) - 1
        ev(dvt, (lambda r0=r0, nr=nr, last=last: d_chunk(r0, nr, last)))
        dvt += cost
        for rr in range(r0, r0 + nr, 4):
            row_ready[rr] = dvt + 1.2

    # Pool: add trees, serial, gated by the DP muls.
    pvt = 0.0
    for r0, nr in DP_CHUNKS:
        pvt = max(pvt, dp_mul_done[r0])
        ev(pvt, (lambda r0=r0, nr=nr: dp_adds(r0, nr)))
        pvt += 8 * PADD_C(nr * W)
        for rr in range(r0, r0 + nr, 4):
            row_ready[rr] = pvt + 1.2

    # 1x1 spans at max over their two D tiles' readiness.  Late spans use
    # DVE for the bias evac (Act is the convoy then) and the Pool DMA queue
    # for the final output pairs.
    for s in range(16):
        rt = max(row_ready[8 * s], row_ready[8 * s + 4])
        ev(rt, (lambda s=s, rt=rt: span_1x1(s, evac_dve=(s == 15))))

    dp_p3 = {}

    def dp_adds(r0, nr):
        adds_and_lrelu(nc.gpsimd, dp_p3[r0], r0, nr, "pacc")

    def d_chunk(r0, nr, last=False):
        p3 = dve_muls(r0, nr, dprod)
        adds_and_lrelu(nc.vector, p3, r0, nr, "dacc", lrelu_dve=last)

    for _, _, fn in sorted(events, key=lambda e: (e[0], e[1])):
        fn()


# ---------------------------------------------------------------------------
# host-side entry point
# ---------------------------------------------------------------------------

_PROGRAM_CACHE: dict[str, bass.Bass] = {}


def _get_program() -> bass.Bass:
    if "p" not in _PROGRAM_CACHE:
        _PROGRAM_CACHE["p"] = build_program()
    return _PROGRAM_CACHE["p"]


def _host_prep(inputs: dict):
    import ml_dtypes

    x = np.asarray(inputs["x"], dtype=np.float32)
    d = np.asarray(inputs["d"], dtype=np.float32)
    Wk1 = np.asarray(inputs["Wk1"], dtype=np.float32)
    Wk2 = np.asarray(inputs["Wk2"], dtype=np.float32)
    Wc = np.asarray(inputs["Wc"], dtype=np.float32)
    bc = np.asarray(inputs["bc"], dtype=np.float32)

    wk1t = np.ascontiguousarray(Wk1.T)
    w = Wk2.reshape(C, KK * KK, C).transpose(2, 1, 0)  # (j, t, c)
    wk2td = np.concatenate([w, w], axis=2).reshape(C, KK * KK * 2 * C)
    wcb = np.zeros((2 * C, 2 * C), dtype=np.float32)
    wcb[0:C, 0:C] = Wc.T
    wcb[C:, C:] = Wc.T
    wcb = wcb.astype(ml_dtypes.bfloat16)
    bc2 = np.ascontiguousarray(np.concatenate([bc, bc]).reshape(2 * C, 1))

    # host-side zero-padding: [S*C, RP, RS] with image at [1:H+1, 1:W+1]
    B = x.shape[0]
    xpad = np.zeros((B, C, RP, RS), dtype=ml_dtypes.bfloat16)
    xpad[:, :, 1 : H + 1, 1 : W + 1] = x.astype(ml_dtypes.bfloat16)

    in_maps = []
    for i in range(NCORES):
        xp = np.ascontiguousarray(
            xpad[S * i : S * (i + 1)].reshape(S * C, XFREE)
        )
        dT = d[S * i : S * (i + 1)].T
        smalls = np.ascontiguousarray(
            np.concatenate([dT, wk1t, wk2td], axis=1)
        ).astype(ml_dtypes.bfloat16)
        in_maps.append(
            {
                "xpad": xp,
                "smalls": smalls,
                "wcb": wcb,
                "bc2": bc2,
            }
        )
    return in_maps


def run_on_hw(inputs: dict, **kwargs):
    """Run the SPMD kernel on 8 NeuronCores; returns (output, results)."""
    from concourse.bass_utils import run_bass_kernel_spmd

    nc = _get_program()
    in_maps = _host_prep(inputs)
    res = run_bass_kernel_spmd(nc, in_maps, core_ids=list(range(NCORES)), **kwargs)
    outs = res.results
    B = S * NCORES
    out = np.empty((B, C, H, W), dtype=np.float32)
    for i in range(NCORES):
        out[S * i : S * (i + 1)] = outs[i]["out"].astype(np.float32).reshape(
            S, C, H, W
        )
    return out, res


def kernel(**inputs) -> np.ndarray:
    out, _ = run_on_hw(inputs)
    return out


if __name__ == "__main__":
    nc = build_program()
    print("program built OK")
